# revision 11
# baseline (speedup 1.0000x reference)
"""BlockGRUCell Trainium2 kernel.

Computation (per reference):
  hx = concat([h, x], -1)                       # (B, 2048)
  gate[b, 192g+o] = sum_i hx[b, 128g+i] * W[g, o, i]   # block-diagonal matmul
  r, c, u = split(gate + bias, 3)               # bias == 0 from setup_inputs
  h_new = sigmoid(u) * tanh(sigmoid(r) * c) + (1 - sigmoid(u)) * h

Sharding: data-parallel over batch across 8 NeuronCores (2048 rows each),
weights replicated.

The TensorE matmul contracts over the partition dim, so the stationary
operand must be hx^T per 128-feature block. Rather than transposing on
device (PE transpose + PSUM->SBUF cast ate the VectorE budget and stalled
the PE), the host pre-packs x and h into per-tile transposed bf16 panels:
  xt_tiled[t, p, 128g+b] = x[128t+b, 128g+p]
so each 128-row batch tile's matmul operands arrive as one contiguous DMA.

Per core, per 128-row tile:
  - DMA: ht_t, xt_t (bf16 transposed panels), h_t (fp32 natural)
  - 16 block matmuls (bf16, fp32 accum) into a [128, 3072] PSUM gate panel
    split into two 3-bank halves (so the next tile's matmuls only wait on
    the early epilogue reads); matmuls split at PSUM bank crossings
  - ScalarE: sigmoid(r), tanh(reset*c), sigmoid(u)
  - VectorE: reset*c from PSUM, upd*cand, final add
  - GpSimd:  (1-upd), (1-upd)*h  (off the tanh critical chain)
"""

import numpy as np
import ml_dtypes

import concourse.bass as bass
import concourse.bacc as bacc
import concourse.tile as tile
import concourse.mybir as mybir
from concourse.bass_utils import run_bass_kernel_spmd

N_CORES = 8
BATCH = 16384
BS = BATCH // N_CORES            # rows per core
P = 128
NT = BS // P                     # 128-row tiles per core
HID = 1024
G = 16                           # feature blocks
IN_PER = 128
OUT_PER = 192
GATE = 3 * HID                   # 3072
HALF = GATE // 2                 # 1536 cols = 3 PSUM banks
PSUM_BANK_F32 = 512

F32 = mybir.dt.float32
BF16 = mybir.dt.bfloat16
AFT = mybir.ActivationFunctionType


def _body(tc, nc, xt_d, ht_d, h_d, wt_d, out_d):
    with (
        tc.tile_pool(name="consts", bufs=1) as consts,
        tc.tile_pool(name="io", bufs=4) as io,
        tc.tile_pool(name="panels", bufs=3) as panels,
        tc.tile_pool(name="gatep", bufs=2, space="PSUM") as gatep,
    ):
        wt_s = consts.tile([P, G * OUT_PER], BF16)
        nc.sync.dma_start(out=wt_s, in_=wt_d[:, :])

        ht_t = io.tile([P, 8 * P], BF16, tag="ht")
        xt_t = io.tile([P, 8 * P], BF16, tag="xt")
        h_t = io.tile([P, HID], F32, tag="h")
        nc.sync.dma_start(out=ht_t, in_=ht_d[0])
        nc.sync.dma_start(out=xt_t, in_=xt_d[0])
        nc.sync.dma_start(out=h_t, in_=h_d[0:P, :])

        for t in range(NT):
            if t > 0:
                ht_t = io.tile([P, 8 * P], BF16, tag="ht")
                xt_t = io.tile([P, 8 * P], BF16, tag="xt")
                h_t = io.tile([P, HID], F32, tag="h")
                nc.sync.dma_start(out=ht_t, in_=ht_d[t])
                nc.sync.dma_start(out=xt_t, in_=xt_d[t])
                nc.sync.dma_start(out=h_t, in_=h_d[t * P:(t + 1) * P, :])

            # gate panel split in two 3-bank halves
            gA = gatep.tile([P, HALF], F32, tag="gate")   # blocks 0..7
            gB = gatep.tile([P, HALF], F32, tag="gate")   # blocks 8..15

            for g in range(G):
                lhsT = ht_t[:, (g % 8) * P:(g % 8 + 1) * P] if g < 8 else \
                       xt_t[:, (g - 8) * P:(g - 7) * P]
                gate, c0 = (gA, g * OUT_PER) if g < 8 else \
                           (gB, g * OUT_PER - HALF)
                c1 = c0 + OUT_PER
                # a matmul output may not cross a PSUM bank boundary
                mid = ((c0 // PSUM_BANK_F32) + 1) * PSUM_BANK_F32
                w0 = g * OUT_PER
                if c1 <= mid:
                    nc.tensor.matmul(gate[:, c0:c1], lhsT,
                                     wt_s[:, w0:w0 + OUT_PER],
                                     start=True, stop=True)
                else:
                    nc.tensor.matmul(gate[:, c0:mid], lhsT,
                                     wt_s[:, w0:w0 + mid - c0],
                                     start=True, stop=True)
                    nc.tensor.matmul(gate[:, mid:c1], lhsT,
                                     wt_s[:, w0 + mid - c0:w0 + OUT_PER],
                                     start=True, stop=True)

            # epilogue: r = gate[0:1024], c = gate[1024:2048], u = [2048:3072]
            # gA = cols [0:1536), gB = cols [1536:3072)
            reset = panels.tile([P, HID], F32, tag="reset")
            nc.scalar.activation(reset, gA[:, 0:HID], AFT.Sigmoid)
            rc = panels.tile([P, HID], F32, tag="rc")
            nc.vector.tensor_tensor(rc[:, 0:HALF - HID], gA[:, HID:HALF],
                                    reset[:, 0:HALF - HID],
                                    mybir.AluOpType.mult)
            nc.vector.tensor_tensor(rc[:, HALF - HID:HID],
                                    gB[:, 0:2 * HID - HALF],
                                    reset[:, HALF - HID:HID],
                                    mybir.AluOpType.mult)
            cand = panels.tile([P, HID], F32, tag="cand")
            nc.scalar.activation(cand, rc, AFT.Tanh)
            upd = panels.tile([P, HID], F32, tag="upd")
            nc.scalar.activation(upd, gB[:, 2 * HID - HALF:GATE - HALF],
                                 AFT.Sigmoid)

            # h_new = upd*cand + (1-upd)*h, all on VectorE: fp32
            # tensor_tensor is 1x everywhere and GpSimd work would steal
            # DVE's second read port
            u1 = panels.tile([P, HID], F32, tag="u1")
            nc.vector.tensor_scalar(u1, upd, -1.0, 1.0,
                                    op0=mybir.AluOpType.mult,
                                    op1=mybir.AluOpType.add)
            w = panels.tile([P, HID], F32, tag="w")
            nc.vector.tensor_mul(w, u1, h_t)
            v = panels.tile([P, HID], F32, tag="v")
            nc.vector.tensor_mul(v, upd, cand)
            hn = panels.tile([P, HID], F32, tag="hn")
            nc.vector.tensor_add(hn, v, w)
            nc.sync.dma_start(out=out_d[t * P:(t + 1) * P, :], in_=hn)


_NC_CACHE = {}


def _build_nc():
    if "nc" in _NC_CACHE:
        return _NC_CACHE["nc"]
    nc = bacc.Bacc()
    xt_d = nc.dram_tensor("xt", [NT, P, 8 * P], BF16, kind="ExternalInput")
    ht_d = nc.dram_tensor("ht", [NT, P, 8 * P], BF16, kind="ExternalInput")
    h_d = nc.dram_tensor("h", [BS, HID], F32, kind="ExternalInput")
    wt_d = nc.dram_tensor("wt", [P, G * OUT_PER], BF16,
                          kind="ExternalInput")
    out_d = nc.dram_tensor("out", [BS, HID], F32, kind="ExternalOutput")
    with tile.TileContext(nc) as tc:
        _body(tc, nc, xt_d, ht_d, h_d, wt_d, out_d)
    nc.compile()
    _NC_CACHE["nc"] = nc
    return nc


def _np_reference(x, h, weight, bias):
    hx = np.concatenate([h, x], axis=-1)
    xg = hx.reshape(x.shape[0], G, IN_PER)
    gate = np.einsum("bgi,goi->bgo", xg, weight).reshape(x.shape[0], GATE)
    gate = gate + bias
    r, c, u = np.split(gate, 3, axis=-1)
    reset = 1.0 / (1.0 + np.exp(-r))
    cand = np.tanh(reset * c)
    upd = 1.0 / (1.0 + np.exp(-u))
    return (upd * cand + (1.0 - upd) * h).astype(np.float32)


def _pack_transposed(a):
    """[BS, 1024] fp32 -> [NT, 128, 1024] bf16 with
    out[t, p, 128g+b] = a[128t+b, 128g+p]."""
    t = a.reshape(NT, P, 8, P).transpose(0, 3, 2, 1)     # [t, p, g, b]
    return np.ascontiguousarray(t.reshape(NT, P, 8 * P)).astype(
        ml_dtypes.bfloat16)


def _run(x, h, weight, bias, trace=False, tmpdir=None):
    # wt[p, 192g+o] = W[g, o, p] — the exact SBUF layout, one contiguous DMA
    wt = np.ascontiguousarray(
        weight.transpose(2, 0, 1).reshape(P, G * OUT_PER)).astype(
        ml_dtypes.bfloat16)
    nc = _build_nc()
    in_maps = []
    for c in range(N_CORES):
        sl = slice(c * BS, (c + 1) * BS)
        xs, hs = x[sl], h[sl]
        in_maps.append({
            "xt": _pack_transposed(xs),
            "ht": _pack_transposed(hs),
            "h": np.ascontiguousarray(hs),
            "wt": wt,
        })
    res = run_bass_kernel_spmd(nc, in_maps, core_ids=list(range(N_CORES)),
                               trace=trace, tmpdir=tmpdir)
    out = np.concatenate([m["out"] for m in res.results], axis=0)
    return out, res


def kernel(x, h, weight, bias):
    x = np.asarray(x, dtype=np.float32)
    h = np.asarray(h, dtype=np.float32)
    weight = np.asarray(weight, dtype=np.float32)
    bias = np.asarray(bias, dtype=np.float32)
    if np.any(bias != 0.0):
        # setup_inputs() always passes zero bias; keep a correct fallback.
        return _np_reference(x, h, weight, bias)
    out, _ = _run(x, h, weight, bias)
    return out


# revision 14
# speedup vs baseline: 1.2891x; 1.2891x over previous
"""BlockGRUCell Trainium2 kernel.

Computation (per reference):
  hx = concat([h, x], -1)                       # (B, 2048)
  gate[b, 192g+o] = sum_i hx[b, 128g+i] * W[g, o, i]   # block-diagonal matmul
  r, c, u = split(gate + bias, 3)               # bias == 0 from setup_inputs
  h_new = sigmoid(u) * tanh(sigmoid(r) * c) + (1 - sigmoid(u)) * h

Sharding: data-parallel over batch across 8 NeuronCores (2048 rows each),
weights replicated.

The TensorE matmul contracts over the partition dim, so the stationary
operand must be hx^T per 128-feature block. Rather than transposing on
device (PE transpose + PSUM->SBUF cast ate the VectorE budget and stalled
the PE), the host pre-packs x and h into per-tile transposed bf16 panels:
  xt_tiled[t, p, 128g+b] = x[128t+b, 128g+p]
so each 128-row batch tile's matmul operands arrive as one contiguous DMA.

Per core, per 128-row tile:
  - DMA: ht_t, xt_t (bf16 transposed panels), h_t (fp32 natural)
  - 16 block matmuls (bf16, fp32 accum) into a [128, 3072] PSUM gate panel
    split into two 3-bank halves (so the next tile's matmuls only wait on
    the early epilogue reads); matmuls split at PSUM bank crossings
  - ScalarE: sigmoid(r), tanh(reset*c), sigmoid(u)
  - VectorE: reset*c from PSUM, upd*cand, final add
  - GpSimd:  (1-upd), (1-upd)*h  (off the tanh critical chain)
"""

import numpy as np
import ml_dtypes

import concourse.bass as bass
import concourse.bacc as bacc
import concourse.tile as tile
import concourse.mybir as mybir
from concourse.bass_utils import run_bass_kernel_spmd

N_CORES = 8
BATCH = 16384
BS = BATCH // N_CORES            # rows per core
P = 128
NT = BS // P                     # 128-row tiles per core
HID = 1024
G = 16                           # feature blocks
IN_PER = 128
OUT_PER = 192
GATE = 3 * HID                   # 3072
HALF = GATE // 2                 # 1536 cols = 3 PSUM banks
PSUM_BANK_F32 = 512

F32 = mybir.dt.float32
BF16 = mybir.dt.bfloat16
AFT = mybir.ActivationFunctionType


def _body(tc, nc, xt_d, ht_d, h_d, wt_d, out_d):
    with (
        tc.tile_pool(name="consts", bufs=1) as consts,
        tc.tile_pool(name="io", bufs=6) as io,
        tc.tile_pool(name="panels", bufs=4) as panels,
        tc.tile_pool(name="gatep", bufs=4, space="PSUM") as gatep,
    ):
        wt_s = consts.tile([P, G * OUT_PER], BF16)
        nc.sync.dma_start(out=wt_s, in_=wt_d[:, :])

        ht_t = io.tile([P, 8 * P], BF16, tag="ht")
        xt_t = io.tile([P, 8 * P], BF16, tag="xt")
        h_t = io.tile([P, HID], F32, tag="h")
        nc.sync.dma_start(out=ht_t, in_=ht_d[0])
        nc.sync.dma_start(out=xt_t, in_=xt_d[0])
        nc.sync.dma_start(out=h_t, in_=h_d[0:P, :])

        for t in range(NT):
            if t > 0:
                ht_t = io.tile([P, 8 * P], BF16, tag="ht")
                xt_t = io.tile([P, 8 * P], BF16, tag="xt")
                h_t = io.tile([P, HID], F32, tag="h")
                nc.sync.dma_start(out=ht_t, in_=ht_d[t])
                nc.sync.dma_start(out=xt_t, in_=xt_d[t])
                nc.sync.dma_start(out=h_t, in_=h_d[t * P:(t + 1) * P, :])

            # gate panel as three [128, 1024] PSUM tensors = the r/c/u
            # panels exactly (2 banks each; bufs=4 leaves one extra slot so
            # the next tile's r-matmuls can start early)
            gR = gatep.tile([P, HID], F32, tag="gate")
            gC = gatep.tile([P, HID], F32, tag="gate")
            gU = gatep.tile([P, HID], F32, tag="gate")
            gs = (gR, gC, gU)

            for g in range(G):
                lhsT = ht_t[:, (g % 8) * P:(g % 8 + 1) * P] if g < 8 else \
                       xt_t[:, (g - 8) * P:(g - 7) * P]
                w0 = g * OUT_PER
                # split matmul writes at PSUM bank (512) and panel (1024)
                # boundaries
                c0 = w0
                while c0 < w0 + OUT_PER:
                    c1 = min(w0 + OUT_PER,
                             (c0 // PSUM_BANK_F32 + 1) * PSUM_BANK_F32)
                    gate = gs[c0 // HID]
                    nc.tensor.matmul(gate[:, c0 % HID:(c0 % HID) + c1 - c0],
                                     lhsT, wt_s[:, c0:c1],
                                     start=True, stop=True)
                    c0 = c1

            reset = panels.tile([P, HID], F32, tag="reset")
            nc.scalar.activation(reset, gR, AFT.Sigmoid)
            rc = panels.tile([P, HID], F32, tag="rc")
            nc.vector.tensor_tensor(rc, gC, reset, mybir.AluOpType.mult)
            cand = panels.tile([P, HID], F32, tag="cand")
            nc.scalar.activation(cand, rc, AFT.Tanh)
            upd = panels.tile([P, HID], F32, tag="upd")
            nc.scalar.activation(upd, gU, AFT.Sigmoid)

            # h_new = h + upd*(cand - h), all on VectorE: fp32
            # tensor_tensor is 1x everywhere and GpSimd work would steal
            # DVE's second read port
            dd = panels.tile([P, HID], F32, tag="dd")
            nc.vector.tensor_sub(dd, cand, h_t)
            ee = panels.tile([P, HID], F32, tag="ee")
            nc.vector.tensor_mul(ee, upd, dd)
            hn = panels.tile([P, HID], F32, tag="hn")
            nc.vector.tensor_add(hn, h_t, ee)
            nc.sync.dma_start(out=out_d[t * P:(t + 1) * P, :], in_=hn)


_NC_CACHE = {}


def _build_nc():
    if "nc" in _NC_CACHE:
        return _NC_CACHE["nc"]
    nc = bacc.Bacc()
    xt_d = nc.dram_tensor("xt", [NT, P, 8 * P], BF16, kind="ExternalInput")
    ht_d = nc.dram_tensor("ht", [NT, P, 8 * P], BF16, kind="ExternalInput")
    h_d = nc.dram_tensor("h", [BS, HID], F32, kind="ExternalInput")
    wt_d = nc.dram_tensor("wt", [P, G * OUT_PER], BF16,
                          kind="ExternalInput")
    out_d = nc.dram_tensor("out", [BS, HID], F32, kind="ExternalOutput")
    with tile.TileContext(nc) as tc:
        _body(tc, nc, xt_d, ht_d, h_d, wt_d, out_d)
    nc.compile()
    _NC_CACHE["nc"] = nc
    return nc


def _np_reference(x, h, weight, bias):
    hx = np.concatenate([h, x], axis=-1)
    xg = hx.reshape(x.shape[0], G, IN_PER)
    gate = np.einsum("bgi,goi->bgo", xg, weight).reshape(x.shape[0], GATE)
    gate = gate + bias
    r, c, u = np.split(gate, 3, axis=-1)
    reset = 1.0 / (1.0 + np.exp(-r))
    cand = np.tanh(reset * c)
    upd = 1.0 / (1.0 + np.exp(-u))
    return (upd * cand + (1.0 - upd) * h).astype(np.float32)


def _pack_transposed(a):
    """[BS, 1024] fp32 -> [NT, 128, 1024] bf16 with
    out[t, p, 128g+b] = a[128t+b, 128g+p]."""
    t = a.reshape(NT, P, 8, P).transpose(0, 3, 2, 1)     # [t, p, g, b]
    return np.ascontiguousarray(t.reshape(NT, P, 8 * P)).astype(
        ml_dtypes.bfloat16)


def _run(x, h, weight, bias, trace=False, tmpdir=None):
    # wt[p, 192g+o] = W[g, o, p] — the exact SBUF layout, one contiguous DMA
    wt = np.ascontiguousarray(
        weight.transpose(2, 0, 1).reshape(P, G * OUT_PER)).astype(
        ml_dtypes.bfloat16)
    nc = _build_nc()
    in_maps = []
    for c in range(N_CORES):
        sl = slice(c * BS, (c + 1) * BS)
        xs, hs = x[sl], h[sl]
        in_maps.append({
            "xt": _pack_transposed(xs),
            "ht": _pack_transposed(hs),
            "h": np.ascontiguousarray(hs),
            "wt": wt,
        })
    res = run_bass_kernel_spmd(nc, in_maps, core_ids=list(range(N_CORES)),
                               trace=trace, tmpdir=tmpdir)
    out = np.concatenate([m["out"] for m in res.results], axis=0)
    return out, res


def kernel(x, h, weight, bias):
    x = np.asarray(x, dtype=np.float32)
    h = np.asarray(h, dtype=np.float32)
    weight = np.asarray(weight, dtype=np.float32)
    bias = np.asarray(bias, dtype=np.float32)
    if np.any(bias != 0.0):
        # setup_inputs() always passes zero bias; keep a correct fallback.
        return _np_reference(x, h, weight, bias)
    out, _ = _run(x, h, weight, bias)
    return out
